# revision 9
# baseline (speedup 1.0000x reference)
"""LocallyConnected1d Bass kernel for 8 TRN2 NeuronCores — w-stationary scheme.

Problem: x [64, 64, 512] f32, weight [1, 64, 64, 504, 9] f32
         out[b, o, l] = sum_{i,k} x[b, i, l+k] * weight[0, o, i, l, k]
L_out = 504 sharded 8 x 63.  All tensors travel as bf16; host packs/unpacks.

Design (per core) — weights are the STATIONARY operand, x batch columns are
the moving operand:
  - Positions pair up as (l, l+1), l even ("pair" gp = l/2, 31 full pairs +
    the lone position 62).  The PE output tile is [ (t, o) = 128, b = 64 ]:
    t in {0,1} selects the position within the pair, o = C_out.
  - Contraction is (s', i): two consecutive x columns stacked on partitions
    (top = even col q, bottom = q+1) x C_in.  One pass covers 2 x-cols x
    2 positions = 4 kernel taps; 5 passes cover the 9-tap window of both
    positions (two corner blocks are structural zeros).
  - x ships once as [128, 36*64]: unit m = x cols (2m, 2m+1) stacked — a
    pure reshape, no duplication (passes use even-aligned column pairs).
    4608 B/partition replaces the baseline's 17920 B/partition block-diag
    x tiles; that is where the DMA-ring time goes.
  - Weights ship pre-packed as per-pass stationary tiles in PE consumption
    order: 31*5 tiles [128,128] + 5 lone tiles [128,64].
  - PSUM tiles are split so no copy ever reads a tile that later matmuls
    write (the tile framework serializes at tile granularity): pairs 0-7 /
    8-15 / 16-23 in three [128,512] banks, pairs 24-27 in [128,256], and
    pairs 28-30 + the lone position share one [128,256] tile so a single
    DVE copy (one semaphore) gates the final out-DMA.
  - Tail: ring input ends are near-equal and the last chunks are spread so
    each one's matmul backlog clears inside the next chunk's DMA-completion
    shadow; the last chunk is the lone position's 5th tile alone, so the end
    chain is one DMA-completion lag + 1 matmul + a [128,256] copy + a
    [128,256] DMA + the fixed completion/epilogue.
"""

import numpy as np
import ml_dtypes

B = 64
CI = 64
CO = 64
K = 9
L = 512
L_OUT = 504
N_CORES = 8
LP = L_OUT // N_CORES          # 63 positions per core
NPAIR = 31                     # full position pairs per core
NUNIT = 36                     # x column units (2 cols each)
W_COLS = NPAIR * 640 + 5 * 64       # 20160
X_COLS = NUNIT * 64                 # 2304
OUT_COLS = 32 * CO                  # 2048

SP, ACT, POOL = "sync", "scalar", "gpsimd"

# Input chunks in PE consumption order.
#   ("w", lo, hi): pairs [lo,hi) — 640 cols each
#   ("wt", lo_col, hi_col): raw wt column range
#   ("x", lo_u, hi_u): x units [lo,hi)
LONE0 = NPAIR * 640
CHUNKS = [
    ("w", 0, 1),                    # c0   500
    ("x", 0, 14),                   # c1   691
    ("w", 1, 3),                    # c2   987
    ("x", 14, 36),                  # c3  1086
    ("w", 3, 5),                    # c4   987
    ("w", 5, 7),                    # c5   987
    ("w", 7, 9),                    # c6   987
    ("w", 9, 11),                   # c7   987
    ("w", 11, 13),                  # c8   987
    ("w", 13, 15),                  # c9   987
    ("w", 15, 17),                  # c10  987
    ("w", 17, 19),                  # c11  987
    ("w", 19, 21),                  # c12  987
    ("w", 21, 23),                  # c13  987
    ("w", 23, 25),                  # c14  987
    ("w", 25, 27),                  # c15  987
    ("w", 27, 28),                  # c16  500
    ("w", 28, 29),                  # c17  500
    ("wt", 29 * 640, LONE0 + 256),  # c18: pairs 29-30 + lone tiles 0-3, 1184
    ("wt", LONE0 + 256, W_COLS),    # c19: lone tile 4, 500(floor)
]
# ring -> chunk indices (per-ring order = consumption order).  Ring input
# ends are near-equal; the last five chunks are spread so each one's matmul
# backlog clears inside the next chunk's completion shadow.
RING_PLAN = {
    SP: [0, 3, 6, 9, 12, 15, 19],
    ACT: [1, 4, 7, 10, 13, 18],
    POOL: [2, 5, 8, 11, 14, 16, 17],
}


def _build_bass():
    import concourse.bass as bass
    import concourse.mybir as mybir
    from concourse.tile import TileContext

    dt = mybir.dt.bfloat16
    nc = bass.Bass()

    wt_d = nc.dram_tensor("wt", [128, W_COLS], dt, kind="ExternalInput")
    xd_d = nc.dram_tensor("xd", [128, X_COLS], dt, kind="ExternalInput")
    out_d = nc.dram_tensor("out", [128, OUT_COLS], dt, kind="ExternalOutput")

    with TileContext(nc) as tc:
        with (
            tc.tile_pool(name="wc", bufs=1) as wpool,
            tc.tile_pool(name="xc", bufs=1) as xpool,
            tc.tile_pool(name="ps", bufs=1, space="PSUM") as ppool,
            tc.tile_pool(name="ob", bufs=1) as opool,
        ):
            tiles = {}
            for ci, ch in enumerate(CHUNKS):
                if ch[0] == "w":
                    tiles[ci] = wpool.tile([128, (ch[2] - ch[1]) * 640], dt,
                                           name=f"c{ci}")
                elif ch[0] == "wt":
                    tiles[ci] = wpool.tile([128, ch[2] - ch[1]], dt,
                                           name=f"c{ci}")
                else:
                    tiles[ci] = xpool.tile([128, (ch[2] - ch[1]) * 64], dt,
                                           name=f"c{ci}")

            def issue(ci):
                ch = CHUNKS[ci]
                ring = next(e for e, lst in RING_PLAN.items() if ci in lst)
                eng = getattr(nc, ring)
                if ch[0] == "w":
                    eng.dma_start(out=tiles[ci],
                                  in_=wt_d[:, ch[1] * 640:ch[2] * 640])
                elif ch[0] == "wt":
                    eng.dma_start(out=tiles[ci], in_=wt_d[:, ch[1]:ch[2]])
                else:
                    eng.dma_start(out=tiles[ci],
                                  in_=xd_d[:, ch[1] * 64:ch[2] * 64])

            maxlen = max(len(v) for v in RING_PLAN.values())
            for j in range(maxlen):
                for ring in (SP, ACT, POOL):
                    if j < len(RING_PLAN[ring]):
                        issue(RING_PLAN[ring][j])

            def wslice(gp, s):
                """lhsT tile for pair gp pass s (gp=NPAIR -> lone tile s)."""
                if gp < NPAIR:
                    col, width = gp * 640 + s * 128, 128
                else:
                    col, width = LONE0 + s * 64, 64
                for ci, ch in enumerate(CHUNKS):
                    if ch[0] == "w" and ch[1] * 640 <= col < ch[2] * 640:
                        off = col - ch[1] * 640
                        return tiles[ci][:, off:off + width]
                    if ch[0] == "wt" and ch[1] <= col < ch[2]:
                        off = col - ch[1]
                        return tiles[ci][:, off:off + width]
                raise AssertionError

            def xslice(u):
                for ci, ch in enumerate(CHUNKS):
                    if ch[0] == "x" and ch[1] <= u < ch[2]:
                        off = (u - ch[1]) * 64
                        return tiles[ci][:, off:off + 64]
                raise AssertionError

            out_sb = opool.tile([128, OUT_COLS], dt)
            # separate tiles so copies never WAR-serialize later matmuls
            pb0 = ppool.tile([128, 512], mybir.dt.float32)   # pairs 0-7
            pb1 = ppool.tile([128, 512], mybir.dt.float32)   # pairs 8-15
            pb2 = ppool.tile([128, 512], mybir.dt.float32)   # pairs 16-23
            pb3 = ppool.tile([128, 256], mybir.dt.float32)   # pairs 24-27
            # pairs 28-30 + lone share one tile -> one final copy/wait
            pb4 = ppool.tile([128, 256], mybir.dt.float32)
            nc.vector.memset(pb4[64:128, 192:256], 0.0)

            def outp_of(gp):
                if gp < 8:
                    return pb0[:, gp * 64:(gp + 1) * 64]
                if gp < 16:
                    return pb1[:, (gp - 8) * 64:(gp - 7) * 64]
                if gp < 24:
                    return pb2[:, (gp - 16) * 64:(gp - 15) * 64]
                if gp < 28:
                    return pb3[:, (gp - 24) * 64:(gp - 23) * 64]
                return pb4[:, (gp - 28) * 64:(gp - 27) * 64]

            for gp in range(NPAIR):
                outp = outp_of(gp)
                for s in range(5):
                    nc.tensor.matmul(outp, wslice(gp, s), xslice(gp + s),
                                     start=(s == 0), stop=(s == 4))
                if gp == 7:
                    nc.vector.tensor_copy(out=out_sb[:, 0:512], in_=pb0[:, :])
                elif gp == 15:
                    nc.vector.tensor_copy(out=out_sb[:, 512:1024],
                                          in_=pb1[:, :])
                    nc.sync.dma_start(out=out_d[:, 0:1024],
                                      in_=out_sb[:, 0:1024])
                elif gp == 23:
                    nc.vector.tensor_copy(out=out_sb[:, 1024:1536],
                                          in_=pb2[:, :])
                    nc.scalar.dma_start(out=out_d[:, 1024:1536],
                                        in_=out_sb[:, 1024:1536])
                elif gp == 27:
                    nc.vector.tensor_copy(out=out_sb[:, 1536:1792],
                                          in_=pb3[:, :])
                    nc.scalar.dma_start(out=out_d[:, 1536:1792],
                                        in_=out_sb[:, 1536:1792])


            # lone position 62, fed by the tiny final w chunk
            for s in range(5):
                nc.tensor.matmul(pb4[0:64, 192:256], wslice(NPAIR, s),
                                 xslice(NPAIR + s),
                                 start=(s == 0), stop=(s == 4))
            nc.vector.tensor_copy(out=out_sb[:, 1792:2048],
                                  in_=pb4[:, :])
            nc.sync.dma_start(out=out_d[:, 1792:2048],
                              in_=out_sb[:, 1792:2048])
    _split_multi_waits(nc, mybir)
    return nc


def _split_multi_waits(nc, mybir):
    """This walrus build encodes at most ONE sync wait per instruction;
    hoist extra waits onto single-wait NoOps (semantically identical)."""
    for f in nc.m.functions:
        for bb in f.blocks:
            out = []
            for inst in bb.instructions:
                si = inst.sync_info
                waits = list(si.on_wait) if si is not None and si.on_wait else []
                if len(waits) > 1:
                    for k, w in enumerate(waits[:-1]):
                        out.append(mybir.InstNoOp(
                            name=f"{inst.name}-wsplit{k}",
                            engine=inst.engine,
                            sync_info=mybir.SyncInfo(on_wait=[w], on_update=[]),
                            bass_nofuse=True))
                    inst.sync_info = mybir.SyncInfo(
                        on_wait=[waits[-1]],
                        on_update=list(si.on_update) if si.on_update else [])
                out.append(inst)
            bb.instructions = out


def _prep_inputs(x, weight):
    """Returns list of 8 per-core input dicts {wt, xd} (bf16)."""
    npdt = ml_dtypes.bfloat16
    x = np.asarray(x, np.float32)
    w0 = np.asarray(weight, np.float32)[0]               # [CO, CI, L_OUT, K]
    wtr = np.ascontiguousarray(w0.transpose(2, 3, 1, 0))  # [L_OUT, K, CI, CO]
    xt = np.ascontiguousarray(x.transpose(1, 2, 0))       # [CI, L, B]

    in_maps = []
    for m in range(N_CORES):
        L0 = LP * m
        # full-pair stationaries [31 pairs, 5 passes, 128 rows, 128 cols]
        arr = np.zeros((NPAIR, 5, 128, 128), np.float32)
        ls = L0 + 2 * np.arange(NPAIR)                   # t=0 positions
        for s in range(5):
            for sp in range(2):
                for t in range(2):
                    k = 2 * s + sp - t
                    if 0 <= k <= 8:
                        arr[:, s, 64 * sp:64 * sp + 64, 64 * t:64 * t + 64] \
                            = wtr[ls + t, k]
        # tile (gp, s) occupies cols [(gp*5+s)*128, +128), rows 0:128
        wt = arr.transpose(2, 0, 1, 3).reshape(128, NPAIR * 5 * 128)
        # lone tiles [5, 128 rows, 64 cols]
        lone = np.zeros((5, 128, 64), np.float32)
        for s in range(5):
            for sp in range(2):
                k = 2 * s + sp
                if k <= 8:
                    lone[s, 64 * sp:64 * sp + 64, :] = wtr[L0 + 62, k]
        wl = lone.transpose(1, 0, 2).reshape(128, 5 * 64)
        wt_full = np.concatenate([wt, wl], axis=1).astype(npdt)

        # x units [128, 36*64]: unit u rows 0:64 = x col L0+2u, rows 64:128 =
        # x col L0+2u+1 (zero past L-1)
        xd = np.zeros((128, NUNIT, B), np.float32)
        for u in range(NUNIT):
            c0, c1 = L0 + 2 * u, L0 + 2 * u + 1
            if c0 < L:
                xd[0:64, u] = xt[:, c0]
            if c1 < L:
                xd[64:128, u] = xt[:, c1]
        xd = xd.reshape(128, NUNIT * B).astype(npdt)

        in_maps.append({"wt": np.ascontiguousarray(wt_full),
                        "xd": np.ascontiguousarray(xd)})
    return in_maps


def _decode_outputs(results):
    outs = []
    for r in results:
        v = np.asarray(r["out"]).astype(np.float32)      # [128, 2048]
        # col block 64*gp holds pair gp as [t*64+o rows, b cols]; block 31
        # holds the lone position (t=0 only)
        blk = v.reshape(2, CO, 32, B)                    # [t, o, gp, b]
        t = blk.transpose(3, 1, 2, 0).reshape(B, CO, 64)  # [b, o, l=2gp+t]
        outs.append(t[:, :, :LP])
    return np.concatenate(outs, axis=2).astype(np.float32)


_CACHED_NC = None


def kernel(x, weight):
    global _CACHED_NC
    from concourse.bass_utils import run_bass_kernel_spmd

    if _CACHED_NC is None:
        _CACHED_NC = _build_bass()
    in_maps = _prep_inputs(x, weight)
    res = run_bass_kernel_spmd(_CACHED_NC, in_maps, core_ids=list(range(N_CORES)))
    return _decode_outputs(res.results)
